# revision 1
# baseline (speedup 1.0000x reference)
import os
import sys

import numpy as np

for _p in ("/opt/trn_rl_repo",):
    if _p not in sys.path and os.path.isdir(_p):
        sys.path.append(_p)

N = 1500
A = 64
STD = 0.3
PERSON_IDX = 2
INV2S2 = 1.0 / (2.0 * STD * STD)
SCALE = 2.0 * INV2S2

P = 128
NO = 1536            # padded objects (8 cores x 192)
NCORES = 8
OPC = NO // NCORES   # 192 objects per core
NOC = 256            # per-core object padding (2 tiles of 128)
NT = NOC // P        # 2 object tiles per core
KMAX = 24            # persons per device batch
GP = 8               # persons per matmul group
NG = KMAX // GP      # 4 groups
KK = 6 * GP + 1      # 49 contraction rows
NF = GP * A          # 512 free columns (person-in-group x action)

NEG = -1.0e9
TCLAMP = 16.0        # |t| clamp; any clamped pair has exp(-inv2s2*(16-2)^2) = 0
LNFLOOR = -20000.0   # floor for lnobj/SCALE row (fp16-safe, still exp -> 0)


def _mode():
    return os.environ.get("KERNEL_MM", "fp16hl")


def _hilo(a):
    hi = a.astype(np.float16)
    lo = (a - hi.astype(np.float32)).astype(np.float16)
    return hi, lo


def _obj_arrays(bbox, scores):
    best = scores.max(axis=1)
    idx = scores.argmax(axis=1)
    person = idx == PERSON_IDX
    obj = np.where(person, 0.0, best).astype(np.float32)

    w = bbox[:, 2] - bbox[:, 0]
    h = bbox[:, 3] - bbox[:, 1]
    cx = bbox[:, 0] + 0.5 * w
    cy = bbox[:, 1] + 0.5 * h

    cx_p = np.zeros(NO, np.float32); cx_p[:N] = cx
    cy_p = np.zeros(NO, np.float32); cy_p[:N] = cy
    lw_p = np.zeros(NO, np.float32); lw_p[:N] = np.log(w)
    lh_p = np.zeros(NO, np.float32); lh_p[:N] = np.log(h)
    lnobj_p = np.full(NO, NEG, np.float32)
    pos = obj > 0
    lnobj_p[:N] = np.where(pos, np.log(np.maximum(obj, 1e-38)), NEG)
    return person, best, w, h, cx, cy, cx_p, cy_p, lw_p, lh_p, lnobj_p


def _host_prep(hidx, best, w, h, cx, cy, obj_arr, target_mean, action_logits):
    """Build in_maps for one batch of <=KMAX persons (object-axis sharding)."""
    cx_p, cy_p, lw_p, lh_p, lnobj_p = obj_arr
    k = len(hidx)

    invw = np.ones(KMAX, np.float32); invw[:k] = 1.0 / w[hidx]
    invh = np.ones(KMAX, np.float32); invh[:k] = 1.0 / h[hidx]
    cxh = np.zeros(KMAX, np.float32); cxh[:k] = cx[hidx]
    cyh = np.zeros(KMAX, np.float32); cyh[:k] = cy[hidx]
    lwh = np.zeros(KMAX, np.float32); lwh[:k] = np.log(w[hidx])
    lhh = np.zeros(KMAX, np.float32); lhh[:k] = np.log(h[hidx])
    mu = np.zeros((KMAX, A, 4), np.float32); mu[:k] = target_mean[hidx]
    m2 = (mu * mu).sum(axis=-1)                               # [KMAX, A]
    lh_ = np.zeros((KMAX, A), np.float32)
    lh_[:k] = best[hidx][:, None] * action_logits[hidx]

    # person-side rhs [NG, KK, NF] block-diagonal (same for all cores)
    rhs = np.zeros((NG, KK, NF), np.float32)
    mug = mu.reshape(NG, GP, A, 4)
    m2g = m2.reshape(NG, GP, A)
    for j in range(GP):
        blk = slice(j * A, (j + 1) * A)
        for cc in range(4):
            rhs[:, cc * GP + j, blk] = mug[:, j, :, cc]
        rhs[:, 4 * GP + j, blk] = 1.0
        rhs[:, 5 * GP + j, blk] = -0.5 * m2g[:, j]
    rhs[:, 6 * GP, :] = 1.0

    lrow = lh_.reshape(NG, NF)
    if _mode() == "fp16hl":
        lrep = np.ascontiguousarray(
            np.broadcast_to(lrow[:, None, :], (NG, P, NF))
        )
        bhi, blo = _hilo(rhs)
        rhs_hh = np.concatenate([bhi, bhi], axis=1)         # [NG, 2KK, NF]
        rhs_lo = blo                                        # [NG, KK, NF]
    else:
        lrep = np.ascontiguousarray(
            np.broadcast_to(lrow[:, None, :], (NG, P, NF))
        )
        rhs_hh = rhs_lo = None

    in_maps = []
    for c in range(NCORES):
        sl = slice(c * OPC, (c + 1) * OPC)
        cxo = np.zeros(NOC, np.float32); cxo[:OPC] = cx_p[sl]
        cyo = np.zeros(NOC, np.float32); cyo[:OPC] = cy_p[sl]
        lwo = np.zeros(NOC, np.float32); lwo[:OPC] = lw_p[sl]
        lho = np.zeros(NOC, np.float32); lho[:OPC] = lh_p[sl]
        lno = np.full(NOC, NEG, np.float32); lno[:OPC] = lnobj_p[sl]

        tx = cxo[None, :] * invw[:, None] - (cxh * invw)[:, None]   # [KMAX, NOC]
        ty = cyo[None, :] * invh[:, None] - (cyh * invh)[:, None]
        tw = lwo[None, :] - lwh[:, None]
        th = lho[None, :] - lhh[:, None]
        tx = np.clip(tx, -TCLAMP, TCLAMP)
        ty = np.clip(ty, -TCLAMP, TCLAMP)
        tw = np.clip(tw, -TCLAMP, TCLAMP)
        th = np.clip(th, -TCLAMP, TCLAMP)
        e2 = tx * tx + ty * ty + tw * tw + th * th

        lhsT = np.zeros((NG, KK, NOC), np.float32)
        g4 = lambda a: a.reshape(NG, GP, NOC)
        lhsT[:, 0:GP] = g4(tx)
        lhsT[:, GP:2 * GP] = g4(ty)
        lhsT[:, 2 * GP:3 * GP] = g4(tw)
        lhsT[:, 3 * GP:4 * GP] = g4(th)
        lhsT[:, 4 * GP:5 * GP] = g4(-0.5 * e2)
        lhsT[:, 5 * GP:6 * GP] = 1.0
        lhsT[:, 6 * GP] = np.maximum(lno / SCALE, LNFLOOR)

        if _mode() == "fp16hl":
            ahi, alo = _hilo(lhsT)
            blob = np.zeros((NG, 2 * KK, NOC + 2 * NF), np.float16)
            blob[:, :KK, :NOC] = ahi
            blob[:, KK:, :NOC] = alo
            blob[:, :, NOC:NOC + NF] = rhs_hh
            blob[:, :KK, NOC + NF:] = rhs_lo
            in_maps.append({"blob": blob, "lrep": lrep})
        else:
            in_maps.append({"lhst": lhsT, "rhs": rhs, "lrep": lrep})
    return in_maps


def _gather(results, k):
    parts = []
    for r in results:
        o = np.asarray(r["out"])
        if _mode() == "fp16hl":
            # [NG, P, NT*NF] -> persons x objects x actions
            o = o.reshape(NG, P, NT, GP, A)
            o = o.transpose(0, 3, 2, 1, 4).reshape(KMAX, NOC, A)
        else:
            o = o.reshape(NG, NT, P, GP, A)
            o = o.transpose(0, 3, 1, 2, 4).reshape(KMAX, NOC, A)
        parts.append(o[:k, :OPC, :])
    return np.concatenate(parts, axis=1)[:, :N, :]


_NC_CACHE = {}


def _build_nc():
    if "nc" in _NC_CACHE:
        return _NC_CACHE["nc"]
    import concourse.bacc as bacc
    import concourse.mybir as mybir
    from concourse.tile import TileContext

    f32 = mybir.dt.float32
    f16 = mybir.dt.float16
    mode = _mode()
    mmdt = mybir.dt.float32r if mode == "f32r" else mybir.dt.float32
    nc = bacc.Bacc()
    if mode == "fp16hl":
        WB = NOC + 2 * NF       # 1280 blob cols
        NW = NT * NF            # 1024 output cols
        blob_d = nc.dram_tensor(
            "blob", [NG, 2 * KK, WB], f16, kind="ExternalInput"
        )
        lrep_d = nc.dram_tensor("lrep", [NG, P, NF], f32, kind="ExternalInput")
        out_d = nc.dram_tensor("out", [NG, P, NW], f32, kind="ExternalOutput")

        with TileContext(nc) as tc:
            with (
                tc.tile_pool(name="wts", bufs=3) as wpool,
                tc.tile_pool(name="work", bufs=3) as work,
                tc.tile_pool(name="mmps", bufs=3, space="PSUM") as pspool,
            ):
                for g in range(NG):
                    blob = wpool.tile([2 * KK, WB], f16, tag="blob")
                    nc.sync.dma_start(blob[:], blob_d[g])
                    lrep = wpool.tile([P, NF], f32, tag="lrep")
                    nc.sync.dma_start(lrep[:], lrep_d[g])

                    ps = pspool.tile([P, NW], f32, tag="mm")
                    for t in range(NT):
                        csl = slice(t * NF, (t + 1) * NF)
                        psl = slice(t * P, (t + 1) * P)
                        nc.tensor.matmul(
                            ps[:, csl], blob[:, psl],
                            blob[:, NOC:NOC + NF],
                            start=True, stop=False,
                        )
                        nc.tensor.matmul(
                            ps[:, csl], blob[0:KK, psl],
                            blob[0:KK, NOC + NF:],
                            start=False, stop=True,
                        )
                        ex = work.tile([P, NF], f32, tag="ex")
                        nc.scalar.activation(
                            ex[:], ps[:, csl], mybir.ActivationFunctionType.Exp,
                            scale=float(SCALE),
                        )
                        ot = work.tile([P, NF], f32, tag="ot")
                        nc.vector.tensor_mul(ot[:], ex[:], lrep[:])
                        nc.sync.dma_start(out_d[g][:, csl], ot[:])
        nc.finalize()
        _NC_CACHE["nc"] = nc
        return nc

    lhst_d = nc.dram_tensor("lhst", [NG, KK, NOC], mmdt, kind="ExternalInput")
    rhs_d = nc.dram_tensor("rhs", [NG, KK, NF], mmdt, kind="ExternalInput")
    lrep_d = nc.dram_tensor("lrep", [NG, P, NF], f32, kind="ExternalInput")
    out_d = nc.dram_tensor("out", [NG, NT, P, NF], f32, kind="ExternalOutput")

    with TileContext(nc) as tc:
        with (
            tc.tile_pool(name="wts", bufs=2) as wpool,
            tc.tile_pool(name="work", bufs=4) as work,
            tc.tile_pool(name="mmps", bufs=4, space="PSUM") as pspool,
        ):
            for g in range(NG):
                lhsT = wpool.tile([KK, NOC], mmdt, tag="lhsT")
                nc.sync.dma_start(lhsT[:], lhst_d[g])
                rhs = wpool.tile([KK, NF], mmdt, tag="rhs")
                nc.sync.dma_start(rhs[:], rhs_d[g])
                lrep = wpool.tile([P, NF], f32, tag="lrep")
                nc.sync.dma_start(lrep[:], lrep_d[g])

                for t in range(NT):
                    ps = pspool.tile([P, NF], f32, tag="mm")
                    sl = slice(t * P, (t + 1) * P)
                    nc.tensor.matmul(
                        ps[:], lhsT[:, sl], rhs[:],
                        start=True, stop=True,
                    )
                    ex = work.tile([P, NF], f32, tag="ex")
                    nc.scalar.activation(
                        ex[:], ps[:], mybir.ActivationFunctionType.Exp,
                        scale=float(SCALE),
                    )
                    ot = work.tile([P, NF], f32, tag="ot")
                    nc.vector.tensor_mul(ot[:], ex[:], lrep[:])
                    nc.sync.dma_start(out_d[g, t], ot[:])
    nc.finalize()
    _NC_CACHE["nc"] = nc
    return nc


def _run_sim(in_maps):
    results = []
    for m in in_maps:
        lrep = m["lrep"]
        if _mode() == "fp16hl":
            out = np.zeros((NG, P, NT * NF), np.float32)
            for g in range(NG):
                b = m["blob"][g].astype(np.float32)         # [2KK, WB]
                a = b[:, :NOC]
                bh = b[:, NOC:NOC + NF]
                bl = b[:KK, NOC + NF:]
                mm = a.T @ bh + a[:KK, :].T @ bl            # [NOC, NF]
                ex = np.exp(np.minimum(SCALE * mm, 0.0).astype(np.float32))
                o = ex * lrep[g][:, :NF][0][None, :]        # [NOC, NF]
                out[g] = o.reshape(NT, P, NF).transpose(1, 0, 2).reshape(
                    P, NT * NF
                )
        else:
            out = np.zeros((NG, NT, P, NF), np.float32)
            for g in range(NG):
                mm = m["lhst"][g].T @ m["rhs"][g]
                ex = np.exp(np.minimum(SCALE * mm, 0.0).astype(np.float32))
                o = ex * lrep[g][0][None, :]
                out[g] = o.reshape(NT, P, NF)
        results.append({"out": out})
    return results


def kernel(action_logits, target_mean, bbox, scores):
    action_logits = np.asarray(action_logits, np.float32)
    target_mean = np.asarray(target_mean, np.float32)
    bbox = np.asarray(bbox, np.float32)
    scores = np.asarray(scores, np.float32)

    person, best, w, h, cx, cy, cx_p, cy_p, lw_p, lh_p, lnobj_p = _obj_arrays(
        bbox, scores
    )
    obj_arr = (cx_p, cy_p, lw_p, lh_p, lnobj_p)
    hidx_all = np.where(person)[0]

    full = np.zeros((N, N, A), np.float32)
    kernel.last_run = None
    for b0 in range(0, len(hidx_all), KMAX):
        hidx = hidx_all[b0:b0 + KMAX]
        in_maps = _host_prep(
            hidx, best, w, h, cx, cy, obj_arr, target_mean, action_logits
        )
        if os.environ.get("KERNEL_SIM") == "1":
            results = _run_sim(in_maps)
        else:
            from concourse.bass_utils import run_bass_kernel_spmd
            nc = _build_nc()
            kw = {}
            if os.environ.get("KERNEL_TRACE") == "1":
                kw = dict(trace=True, trace_cores=list(range(NCORES)))
            r = run_bass_kernel_spmd(
                nc, in_maps, core_ids=list(range(NCORES)), **kw
            )
            results = r.results
            kernel.last_run = r
        full[hidx] = _gather(results, len(hidx))
    return full



# revision 3
# speedup vs baseline: 1.3030x; 1.3030x over previous
import os
import sys

import numpy as np

for _p in ("/opt/trn_rl_repo",):
    if _p not in sys.path and os.path.isdir(_p):
        sys.path.append(_p)

N = 1500
A = 64
STD = 0.3
PERSON_IDX = 2
INV2S2 = 1.0 / (2.0 * STD * STD)
SCALE = 2.0 * INV2S2

NCORES = 8
OPC = 188            # objects per core (8*188 = 1504 >= 1500)
NO = OPC * NCORES
KMAX = 24            # person slots per launch
NTILE = 12           # 2 persons per matmul tile
TPB = 4              # tiles per band (row-slots at partitions 0/32/64/96)
NBAND = 3
KROWS = 31           # 10 A-hi + ONE + ln + 10 A-lo + 9 B-lo-compensation
BCOLS = 2 * A        # 128 person-side weight columns (2 persons x 64 actions)
BANDC = BCOLS + OPC  # 316 columns per (tile-slot) band chunk

TCLAMP = 16.0        # |t| clamp; clamped pairs give exp(-inv2s2*(16-1)^2) ~ 0
LNFLOOR = -20000.0   # floor for ln(obj)/SCALE (fp16-safe, exp -> 0)


def _hilo(a):
    hi = a.astype(np.float16)
    lo = (a - hi.astype(np.float32)).astype(np.float16)
    return hi, lo


def _obj_arrays(bbox, scores):
    best = scores.max(axis=1)
    idx = scores.argmax(axis=1)
    person = idx == PERSON_IDX
    obj = np.where(person, 0.0, best).astype(np.float32)

    w = bbox[:, 2] - bbox[:, 0]
    h = bbox[:, 3] - bbox[:, 1]
    cx = bbox[:, 0] + 0.5 * w
    cy = bbox[:, 1] + 0.5 * h

    cx_p = np.zeros(NO, np.float32); cx_p[:N] = cx
    cy_p = np.zeros(NO, np.float32); cy_p[:N] = cy
    lw_p = np.zeros(NO, np.float32); lw_p[:N] = np.log(w)
    lh_p = np.zeros(NO, np.float32); lh_p[:N] = np.log(h)
    lno_p = np.full(NO, LNFLOOR, np.float32)
    pos = obj > 0
    lno_p[:N] = np.where(
        pos, np.maximum(np.log(np.maximum(obj, 1e-38)) / SCALE, LNFLOOR), LNFLOOR
    )
    return person, best, w, h, cx, cy, cx_p, cy_p, lw_p, lh_p, lno_p


def _host_prep(hidx, best, w, h, cx, cy, obj_arr, target_mean):
    """Build per-core blobs [NBAND, 128, BANDC] f16 for one batch of <=KMAX
    persons.  Matmul tile t (= 4*band + slot) covers persons (2t, 2t+1):
      S[(pr,a), o] = sum_c mu_c*enc_c - 0.5*e2 - 0.5*m2 + ln(obj)/SCALE
    device computes exp(SCALE*S); host multiplies by humaness*logits."""
    cx_p, cy_p, lw_p, lh_p, lno_p = obj_arr
    k = len(hidx)

    invw = np.ones(KMAX, np.float32); invw[:k] = 1.0 / w[hidx]
    invh = np.ones(KMAX, np.float32); invh[:k] = 1.0 / h[hidx]
    cxh = np.zeros(KMAX, np.float32); cxh[:k] = cx[hidx] / w[hidx]
    cyh = np.zeros(KMAX, np.float32); cyh[:k] = cy[hidx] / h[hidx]
    lwh = np.zeros(KMAX, np.float32); lwh[:k] = np.log(w[hidx])
    lhh = np.zeros(KMAX, np.float32); lhh[:k] = np.log(h[hidx])
    mu = np.zeros((KMAX, A, 4), np.float32); mu[:k] = target_mean[hidx]
    m2 = (mu * mu).sum(axis=-1)                      # [KMAX, A]

    # encodings for all person-slots x padded objects [KMAX, NO]
    tx = np.clip(cx_p[None, :] * invw[:, None] - cxh[:, None], -TCLAMP, TCLAMP)
    ty = np.clip(cy_p[None, :] * invh[:, None] - cyh[:, None], -TCLAMP, TCLAMP)
    tw = np.clip(lw_p[None, :] - lwh[:, None], -TCLAMP, TCLAMP)
    th = np.clip(lh_p[None, :] - lhh[:, None], -TCLAMP, TCLAMP)
    e2 = tx * tx + ty * ty + tw * tw + th * th

    # A-side (object/streaming) rows [NTILE, KROWS, NO]
    enc = np.stack([tx, ty, tw, th, e2], axis=1)     # [KMAX, 5, NO]
    enc = enc.reshape(NTILE, 10, NO)                 # person-pair tiles
    ehi, elo = _hilo(enc)
    Af = np.zeros((NTILE, KROWS, NO), np.float16)
    Af[:, 0:10] = ehi
    Af[:, 10] = np.float16(1.0)
    Af[:, 11] = lno_p.astype(np.float16)[None, :]
    Af[:, 12:22] = elo
    Af[:, 22:26] = ehi[:, 0:4]                       # tx..th person 0 (hi)
    Af[:, 26:30] = ehi[:, 5:9]                       # tx..th person 1 (hi)
    Af[:, 30] = np.float16(1.0)

    # B-side (person/stationary) weights [NTILE, KROWS, BCOLS]
    muhi, mulo = _hilo(mu)                           # [KMAX, A, 4]
    m2hi, m2lo = _hilo(-0.5 * m2)                    # [KMAX, A]
    Bf = np.zeros((NTILE, KROWS, 2, A), np.float16)
    for pr in range(2):
        mh = muhi[pr::2].reshape(NTILE, A, 4)
        ml = mulo[pr::2].reshape(NTILE, A, 4)
        for c in range(4):
            Bf[:, 5 * pr + c, pr] = mh[:, :, c]
            Bf[:, 22 + 4 * pr + c, pr] = ml[:, :, c]
        Bf[:, 5 * pr + 4, pr] = np.float16(-0.5)
        Bf[:, 10, pr] = m2hi[pr::2].reshape(NTILE, A)
        Bf[:, 30, pr] = m2lo[pr::2].reshape(NTILE, A)
        Bf[:, 11, pr] = np.float16(1.0)
    Bf = Bf.reshape(NTILE, KROWS, BCOLS)
    # rows 12..21 share the hi-row coefficients (A-lo limb x same B)
    Bf[:, 12:22] = Bf[:, 0:10]

    in_maps = []
    for c in range(NCORES):
        blob = np.zeros((NBAND, 128, BANDC), np.float16)
        osl = slice(c * OPC, (c + 1) * OPC)
        for b in range(NBAND):
            for s in range(TPB):
                t = TPB * b + s
                blob[b, 32 * s:32 * s + KROWS, 0:BCOLS] = Bf[t]
                blob[b, 32 * s:32 * s + KROWS, BCOLS:BANDC] = Af[t][:, osl]
        in_maps.append({"blob": blob})
    return in_maps


def _gather(results, hidx, best, action_logits, full):
    k = len(hidx)
    lh_ = best[hidx][:, None] * action_logits[hidx]           # [k, A]
    big = np.stack([np.asarray(r["out"]) for r in results])   # [8,3,128,4,188]
    g = big.reshape(NCORES, NBAND, 2, A, TPB, OPC)
    g = g.transpose(1, 4, 2, 0, 5, 3)                         # b,s,pr,c,o,a
    g = g.reshape(KMAX, NO, A)[:k, :N, :].astype(np.float32)
    full[hidx] = g * lh_[:, None, :]


_NC_CACHE = {}


def _build_nc():
    if "nc" in _NC_CACHE:
        return _NC_CACHE["nc"]
    import concourse.bacc as bacc
    import concourse.mybir as mybir
    from concourse.tile import TileContext

    f32 = mybir.dt.float32
    f16 = mybir.dt.float16
    nc = bacc.Bacc()
    blob_d = nc.dram_tensor(
        "blob", [NBAND, 128, BANDC], f16, kind="ExternalInput"
    )
    out_d = nc.dram_tensor(
        "out", [NBAND, 128, TPB, OPC], f16, kind="ExternalOutput"
    )

    with TileContext(nc) as tc:
        with (
            tc.tile_pool(name="inp", bufs=3) as ip,
            tc.tile_pool(name="ps", bufs=2, space="PSUM") as pp,
            tc.tile_pool(name="ob", bufs=2) as ob,
        ):
            blobs = []
            for b in range(NBAND):
                blob = ip.tile([128, BANDC], f16, tag="blob")
                nc.sync.dma_start(blob[:], blob_d[b])
                blobs.append(blob)
            for b in range(NBAND):
                blob = blobs[b]
                ps = pp.tile([128, TPB, 512], f32, tag="ps")
                for s in range(TPB):
                    nc.tensor.matmul(
                        ps[:, s, 0:OPC],
                        blob[32 * s:32 * s + KROWS, 0:BCOLS],
                        blob[32 * s:32 * s + KROWS, BCOLS:BANDC],
                        start=True, stop=True,
                        tile_position=(32 * s, 0),
                    )
                ot = ob.tile([128, TPB, OPC], f16, tag="ot")
                nc.scalar.activation(
                    ot[:], ps[:, :, 0:OPC],
                    mybir.ActivationFunctionType.Exp, scale=float(SCALE),
                )
                nc.sync.dma_start(out_d[b], ot[:])
    nc.finalize()
    _NC_CACHE["nc"] = nc
    return nc


def _run_sim(in_maps):
    results = []
    for m in in_maps:
        blob = m["blob"]
        out = np.zeros((NBAND, 128, TPB, OPC), np.float16)
        for b in range(NBAND):
            for s in range(TPB):
                Bm = blob[b, 32 * s:32 * s + KROWS, 0:BCOLS].astype(np.float32)
                Am = blob[b, 32 * s:32 * s + KROWS, BCOLS:BANDC].astype(
                    np.float32
                )
                S = Bm.T @ Am
                out[b, :, s, :] = np.exp(
                    np.minimum(SCALE * S, 80.0)
                ).astype(np.float16)
        results.append({"out": out})
    return results


def kernel(action_logits, target_mean, bbox, scores):
    action_logits = np.asarray(action_logits, np.float32)
    target_mean = np.asarray(target_mean, np.float32)
    bbox = np.asarray(bbox, np.float32)
    scores = np.asarray(scores, np.float32)

    person, best, w, h, cx, cy, cx_p, cy_p, lw_p, lh_p, lno_p = _obj_arrays(
        bbox, scores
    )
    obj_arr = (cx_p, cy_p, lw_p, lh_p, lno_p)
    hidx_all = np.where(person)[0]

    full = np.zeros((N, N, A), np.float32)
    kernel.last_run = None
    for b0 in range(0, len(hidx_all), KMAX):
        hidx = hidx_all[b0:b0 + KMAX]
        in_maps = _host_prep(hidx, best, w, h, cx, cy, obj_arr, target_mean)
        if os.environ.get("KERNEL_SIM") == "1":
            results = _run_sim(in_maps)
        else:
            from concourse.bass_utils import run_bass_kernel_spmd
            nc = _build_nc()
            kw = {}
            if os.environ.get("KERNEL_TRACE") == "1":
                kw = dict(trace=True, trace_cores=list(range(NCORES)))
            r = run_bass_kernel_spmd(
                nc, in_maps, core_ids=list(range(NCORES)), **kw
            )
            results = r.results
            kernel.last_run = r
        _gather(results, hidx, best, action_logits, full)
    return full


# revision 7
# speedup vs baseline: 1.4439x; 1.1081x over previous
import os
import sys

import numpy as np

for _p in ("/opt/trn_rl_repo",):
    if _p not in sys.path and os.path.isdir(_p):
        sys.path.append(_p)

N = 1500
A = 64
STD = 0.3
PERSON_IDX = 2
INV2S2 = 1.0 / (2.0 * STD * STD)
SCALE = 2.0 * INV2S2

NCORES = 8
OPC = 188            # objects per core (8*188 = 1504 >= 1500)
NO = OPC * NCORES
KMAX = 24            # person slots per launch
NTILE = 12           # 2 persons per matmul tile
TPB = 4              # tiles per band (row-slots at partitions 0/32/64/96)
NBAND = 3
KROWS = 31           # 10 A-hi + ONE + ln + 10 A-lo + 9 B-lo-compensation
BCOLS = 2 * A        # 128 person-side weight columns (2 persons x 64 actions)
BANDC = BCOLS + OPC  # 316 columns per (tile-slot) band chunk

TCLAMP = 16.0        # |t| clamp; clamped pairs give exp(-inv2s2*(16-1)^2) ~ 0
LNFLOOR = -20000.0   # floor for ln(obj)/SCALE (fp16-safe, exp -> 0)


def _hilo(a):
    hi = a.astype(np.float16)
    lo = (a - hi.astype(np.float32)).astype(np.float16)
    return hi, lo


def _obj_arrays(bbox, scores):
    best = scores.max(axis=1)
    idx = scores.argmax(axis=1)
    person = idx == PERSON_IDX
    obj = np.where(person, 0.0, best).astype(np.float32)

    w = bbox[:, 2] - bbox[:, 0]
    h = bbox[:, 3] - bbox[:, 1]
    cx = bbox[:, 0] + 0.5 * w
    cy = bbox[:, 1] + 0.5 * h

    cx_p = np.zeros(NO, np.float32); cx_p[:N] = cx
    cy_p = np.zeros(NO, np.float32); cy_p[:N] = cy
    lw_p = np.zeros(NO, np.float32); lw_p[:N] = np.log(w)
    lh_p = np.zeros(NO, np.float32); lh_p[:N] = np.log(h)
    lno_p = np.full(NO, LNFLOOR, np.float32)
    pos = obj > 0
    lno_p[:N] = np.where(
        pos, np.maximum(np.log(np.maximum(obj, 1e-38)) / SCALE, LNFLOOR), LNFLOOR
    )
    return person, best, w, h, cx, cy, cx_p, cy_p, lw_p, lh_p, lno_p


def _host_prep(hidx, best, w, h, cx, cy, obj_arr, target_mean):
    """Build per-core blobs [NBAND, 128, BANDC] f16 for one batch of <=KMAX
    persons.  Matmul tile t (= 4*band + slot) covers persons (2t, 2t+1):
      S[(pr,a), o] = sum_c mu_c*enc_c - 0.5*e2 - 0.5*m2 + ln(obj)/SCALE
    device computes exp(SCALE*S); host multiplies by humaness*logits."""
    cx_p, cy_p, lw_p, lh_p, lno_p = obj_arr
    k = len(hidx)

    invw = np.ones(KMAX, np.float32); invw[:k] = 1.0 / w[hidx]
    invh = np.ones(KMAX, np.float32); invh[:k] = 1.0 / h[hidx]
    cxh = np.zeros(KMAX, np.float32); cxh[:k] = cx[hidx] / w[hidx]
    cyh = np.zeros(KMAX, np.float32); cyh[:k] = cy[hidx] / h[hidx]
    lwh = np.zeros(KMAX, np.float32); lwh[:k] = np.log(w[hidx])
    lhh = np.zeros(KMAX, np.float32); lhh[:k] = np.log(h[hidx])
    mu = np.zeros((KMAX, A, 4), np.float32); mu[:k] = target_mean[hidx]
    m2 = (mu * mu).sum(axis=-1)                      # [KMAX, A]

    # encodings for all person-slots x padded objects [KMAX, NO]
    tx = np.clip(cx_p[None, :] * invw[:, None] - cxh[:, None], -TCLAMP, TCLAMP)
    ty = np.clip(cy_p[None, :] * invh[:, None] - cyh[:, None], -TCLAMP, TCLAMP)
    tw = np.clip(lw_p[None, :] - lwh[:, None], -TCLAMP, TCLAMP)
    th = np.clip(lh_p[None, :] - lhh[:, None], -TCLAMP, TCLAMP)
    e2 = tx * tx + ty * ty + tw * tw + th * th

    # A-side (object/streaming) rows [NTILE, KROWS, NO]
    enc = np.stack([tx, ty, tw, th, e2], axis=1)     # [KMAX, 5, NO]
    enc = enc.reshape(NTILE, 10, NO)                 # person-pair tiles
    ehi, elo = _hilo(enc)
    Af = np.zeros((NTILE, KROWS, NO), np.float16)
    Af[:, 0:10] = ehi
    Af[:, 10] = np.float16(1.0)
    Af[:, 11] = lno_p.astype(np.float16)[None, :]
    Af[:, 12:22] = elo
    Af[:, 22:26] = ehi[:, 0:4]                       # tx..th person 0 (hi)
    Af[:, 26:30] = ehi[:, 5:9]                       # tx..th person 1 (hi)
    Af[:, 30] = np.float16(1.0)

    # B-side (person/stationary) weights [NTILE, KROWS, BCOLS]
    muhi, mulo = _hilo(mu)                           # [KMAX, A, 4]
    m2hi, m2lo = _hilo(-0.5 * m2)                    # [KMAX, A]
    Bf = np.zeros((NTILE, KROWS, 2, A), np.float16)
    for pr in range(2):
        mh = muhi[pr::2].reshape(NTILE, A, 4)
        ml = mulo[pr::2].reshape(NTILE, A, 4)
        for c in range(4):
            Bf[:, 5 * pr + c, pr] = mh[:, :, c]
            Bf[:, 22 + 4 * pr + c, pr] = ml[:, :, c]
        Bf[:, 5 * pr + 4, pr] = np.float16(-0.5)
        Bf[:, 10, pr] = m2hi[pr::2].reshape(NTILE, A)
        Bf[:, 30, pr] = m2lo[pr::2].reshape(NTILE, A)
        Bf[:, 11, pr] = np.float16(1.0)
    Bf = Bf.reshape(NTILE, KROWS, BCOLS)
    # rows 12..21 share the hi-row coefficients (A-lo limb x same B)
    Bf[:, 12:22] = Bf[:, 0:10]

    in_maps = []
    for c in range(NCORES):
        blob = np.zeros((NBAND, 128, BANDC), np.float16)
        osl = slice(c * OPC, (c + 1) * OPC)
        for b in range(NBAND):
            for s in range(TPB):
                t = TPB * b + s
                blob[b, 32 * s:32 * s + KROWS, 0:BCOLS] = Bf[t]
                blob[b, 32 * s:32 * s + KROWS, BCOLS:BANDC] = Af[t][:, osl]
        in_maps.append({
            "blob0b": np.ascontiguousarray(blob[0, :, 0:BCOLS]),
            "blob0a": np.ascontiguousarray(blob[0, :, BCOLS:BANDC]),
            "blob12": np.ascontiguousarray(blob[1:]),
        })
    return in_maps


def _gather(results, hidx, best, action_logits, full):
    k = len(hidx)
    lh_ = best[hidx][:, None] * action_logits[hidx]           # [k, A]
    big = np.stack([np.asarray(r["out"]) for r in results])   # [8,3,128,4,188]
    g = big.reshape(NCORES, NBAND, 2, A, TPB, OPC)
    g = g.transpose(1, 4, 2, 0, 5, 3)                         # b,s,pr,c,o,a
    g = g.reshape(KMAX, NO, A)[:k, :N, :].astype(np.float32)
    full[hidx] = g * lh_[:, None, :]


_NC_CACHE = {}


def _build_nc():
    if "nc" in _NC_CACHE:
        return _NC_CACHE["nc"]
    import concourse.bacc as bacc
    import concourse.mybir as mybir
    from concourse.tile import TileContext

    f32 = mybir.dt.float32
    f16 = mybir.dt.float16
    nc = bacc.Bacc()
    blob0b_d = nc.dram_tensor("blob0b", [128, BCOLS], f16, kind="ExternalInput")
    blob0a_d = nc.dram_tensor("blob0a", [128, OPC], f16, kind="ExternalInput")
    blob12_d = nc.dram_tensor(
        "blob12", [NBAND - 1, 128, BANDC], f16, kind="ExternalInput"
    )
    out_d = nc.dram_tensor(
        "out", [NBAND, 128, TPB, OPC], f16, kind="ExternalOutput"
    )

    with TileContext(nc) as tc:
        with (
            tc.tile_pool(name="inp", bufs=3) as ip,
            tc.tile_pool(name="ps", bufs=2, space="PSUM") as pp,
            tc.tile_pool(name="ob", bufs=3) as ob,
        ):
            blobs = []
            for b in range(NBAND):
                blob = ip.tile([128, BANDC], f16, tag="blob")
                if b == 0:
                    nc.scalar.dma_start(blob[:, 0:BCOLS], blob0b_d[:])
                    nc.sync.dma_start(blob[:, BCOLS:BANDC], blob0a_d[:])
                else:
                    nc.sync.dma_start(blob[:], blob12_d[b - 1])
                blobs.append(blob)
            for b in range(NBAND):
                blob = blobs[b]
                ps = pp.tile([128, TPB, 512], f32, tag="ps")
                for s in range(TPB):
                    nc.tensor.matmul(
                        ps[:, s, 0:OPC],
                        blob[32 * s:32 * s + KROWS, 0:BCOLS],
                        blob[32 * s:32 * s + KROWS, BCOLS:BANDC],
                        start=True, stop=True,
                        tile_position=(32 * s, 0),
                    )
                ot = ob.tile([128, TPB, OPC], f16, tag="ot")
                nc.scalar.activation(
                    ot[:], ps[:, :, 0:OPC],
                    mybir.ActivationFunctionType.Exp, scale=float(SCALE),
                )
                nc.sync.dma_start(out_d[b], ot[:])
    nc.finalize()
    _NC_CACHE["nc"] = nc
    return nc


def _run_sim(in_maps):
    results = []
    for m in in_maps:
        b0 = np.concatenate([m["blob0b"], m["blob0a"]], axis=1)
        blob = np.concatenate([b0[None], m["blob12"]], axis=0)
        out = np.zeros((NBAND, 128, TPB, OPC), np.float16)
        for b in range(NBAND):
            for s in range(TPB):
                Bm = blob[b, 32 * s:32 * s + KROWS, 0:BCOLS].astype(np.float32)
                Am = blob[b, 32 * s:32 * s + KROWS, BCOLS:BANDC].astype(
                    np.float32
                )
                S = Bm.T @ Am
                out[b, :, s, :] = np.exp(
                    np.minimum(SCALE * S, 80.0)
                ).astype(np.float16)
        results.append({"out": out})
    return results


def kernel(action_logits, target_mean, bbox, scores):
    action_logits = np.asarray(action_logits, np.float32)
    target_mean = np.asarray(target_mean, np.float32)
    bbox = np.asarray(bbox, np.float32)
    scores = np.asarray(scores, np.float32)

    person, best, w, h, cx, cy, cx_p, cy_p, lw_p, lh_p, lno_p = _obj_arrays(
        bbox, scores
    )
    obj_arr = (cx_p, cy_p, lw_p, lh_p, lno_p)
    hidx_all = np.where(person)[0]

    full = np.zeros((N, N, A), np.float32)
    kernel.last_run = None
    for b0 in range(0, len(hidx_all), KMAX):
        hidx = hidx_all[b0:b0 + KMAX]
        in_maps = _host_prep(hidx, best, w, h, cx, cy, obj_arr, target_mean)
        if os.environ.get("KERNEL_SIM") == "1":
            results = _run_sim(in_maps)
        else:
            from concourse.bass_utils import run_bass_kernel_spmd
            nc = _build_nc()
            kw = {}
            if os.environ.get("KERNEL_TRACE") == "1":
                kw = dict(trace=True, trace_cores=list(range(NCORES)))
            r = run_bass_kernel_spmd(
                nc, in_maps, core_ids=list(range(NCORES)), **kw
            )
            results = r.results
            kernel.last_run = r
        _gather(results, hidx, best, action_logits, full)
    return full


# revision 9
# speedup vs baseline: 1.4833x; 1.0273x over previous
import os
import sys

import numpy as np

for _p in ("/opt/trn_rl_repo",):
    if _p not in sys.path and os.path.isdir(_p):
        sys.path.append(_p)

N = 1500
A = 64
STD = 0.3
PERSON_IDX = 2
INV2S2 = 1.0 / (2.0 * STD * STD)
SCALE = 2.0 * INV2S2

NCORES = 8
OPC = 188            # objects per core (8*188 = 1504 >= 1500)
NO = OPC * NCORES
KMAX = 24            # person slots per launch
NTILE = 12           # 2 persons per matmul tile
TPB = 4              # tiles per band (row-slots at partitions 0/32/64/96)
NBAND = 3
KROWS = 31           # 10 A-hi + ONE + ln + 10 A-lo + 9 B-lo-compensation
BCOLS = 2 * A        # 128 person-side weight columns (2 persons x 64 actions)
BANDC = BCOLS + OPC  # 316 columns per (tile-slot) band chunk

TCLAMP = 16.0        # |t| clamp; clamped pairs give exp(-inv2s2*(16-1)^2) ~ 0
LNFLOOR = -20000.0   # floor for ln(obj)/SCALE (fp16-safe, exp -> 0)


def _hilo(a):
    hi = a.astype(np.float16)
    lo = (a - hi.astype(np.float32)).astype(np.float16)
    return hi, lo


def _obj_arrays(bbox, scores):
    best = scores.max(axis=1)
    idx = scores.argmax(axis=1)
    person = idx == PERSON_IDX
    obj = np.where(person, 0.0, best).astype(np.float32)

    w = bbox[:, 2] - bbox[:, 0]
    h = bbox[:, 3] - bbox[:, 1]
    cx = bbox[:, 0] + 0.5 * w
    cy = bbox[:, 1] + 0.5 * h

    cx_p = np.zeros(NO, np.float32); cx_p[:N] = cx
    cy_p = np.zeros(NO, np.float32); cy_p[:N] = cy
    lw_p = np.zeros(NO, np.float32); lw_p[:N] = np.log(w)
    lh_p = np.zeros(NO, np.float32); lh_p[:N] = np.log(h)
    lno_p = np.full(NO, LNFLOOR, np.float32)
    pos = obj > 0
    lno_p[:N] = np.where(
        pos, np.maximum(np.log(np.maximum(obj, 1e-38)) / SCALE, LNFLOOR), LNFLOOR
    )
    return person, best, w, h, cx, cy, cx_p, cy_p, lw_p, lh_p, lno_p


def _host_prep(hidx, best, w, h, cx, cy, obj_arr, target_mean):
    """Build per-core blobs [NBAND, 128, BANDC] f16 for one batch of <=KMAX
    persons.  Matmul tile t (= 4*band + slot) covers persons (2t, 2t+1):
      S[(pr,a), o] = sum_c mu_c*enc_c - 0.5*e2 - 0.5*m2 + ln(obj)/SCALE
    device computes exp(SCALE*S); host multiplies by humaness*logits."""
    cx_p, cy_p, lw_p, lh_p, lno_p = obj_arr
    k = len(hidx)

    invw = np.ones(KMAX, np.float32); invw[:k] = 1.0 / w[hidx]
    invh = np.ones(KMAX, np.float32); invh[:k] = 1.0 / h[hidx]
    cxh = np.zeros(KMAX, np.float32); cxh[:k] = cx[hidx] / w[hidx]
    cyh = np.zeros(KMAX, np.float32); cyh[:k] = cy[hidx] / h[hidx]
    lwh = np.zeros(KMAX, np.float32); lwh[:k] = np.log(w[hidx])
    lhh = np.zeros(KMAX, np.float32); lhh[:k] = np.log(h[hidx])
    mu = np.zeros((KMAX, A, 4), np.float32); mu[:k] = target_mean[hidx]
    m2 = (mu * mu).sum(axis=-1)                      # [KMAX, A]

    # encodings for all person-slots x padded objects [KMAX, NO]
    tx = np.clip(cx_p[None, :] * invw[:, None] - cxh[:, None], -TCLAMP, TCLAMP)
    ty = np.clip(cy_p[None, :] * invh[:, None] - cyh[:, None], -TCLAMP, TCLAMP)
    tw = np.clip(lw_p[None, :] - lwh[:, None], -TCLAMP, TCLAMP)
    th = np.clip(lh_p[None, :] - lhh[:, None], -TCLAMP, TCLAMP)
    e2 = tx * tx + ty * ty + tw * tw + th * th

    # A-side (object/streaming) rows [NTILE, KROWS, NO]
    enc = np.stack([tx, ty, tw, th, e2], axis=1)     # [KMAX, 5, NO]
    enc = enc.reshape(NTILE, 10, NO)                 # person-pair tiles
    ehi, elo = _hilo(enc)
    Af = np.zeros((NTILE, KROWS, NO), np.float16)
    Af[:, 0:10] = ehi
    Af[:, 10] = np.float16(1.0)
    Af[:, 11] = lno_p.astype(np.float16)[None, :]
    Af[:, 12:22] = elo
    Af[:, 22:26] = ehi[:, 0:4]                       # tx..th person 0 (hi)
    Af[:, 26:30] = ehi[:, 5:9]                       # tx..th person 1 (hi)
    Af[:, 30] = np.float16(1.0)

    # B-side (person/stationary) weights [NTILE, KROWS, BCOLS]
    muhi, mulo = _hilo(mu)                           # [KMAX, A, 4]
    m2hi, m2lo = _hilo(-0.5 * m2)                    # [KMAX, A]
    Bf = np.zeros((NTILE, KROWS, 2, A), np.float16)
    for pr in range(2):
        mh = muhi[pr::2].reshape(NTILE, A, 4)
        ml = mulo[pr::2].reshape(NTILE, A, 4)
        for c in range(4):
            Bf[:, 5 * pr + c, pr] = mh[:, :, c]
            Bf[:, 22 + 4 * pr + c, pr] = ml[:, :, c]
        Bf[:, 5 * pr + 4, pr] = np.float16(-0.5)
        Bf[:, 10, pr] = m2hi[pr::2].reshape(NTILE, A)
        Bf[:, 30, pr] = m2lo[pr::2].reshape(NTILE, A)
        Bf[:, 11, pr] = np.float16(1.0)
    Bf = Bf.reshape(NTILE, KROWS, BCOLS)
    # rows 12..21 share the hi-row coefficients (A-lo limb x same B)
    Bf[:, 12:22] = Bf[:, 0:10]

    in_maps = []
    for c in range(NCORES):
        blob = np.zeros((NBAND, 128, BANDC), np.float16)
        osl = slice(c * OPC, (c + 1) * OPC)
        for b in range(NBAND):
            for s in range(TPB):
                t = TPB * b + s
                blob[b, 32 * s:32 * s + KROWS, 0:BCOLS] = Bf[t]
                blob[b, 32 * s:32 * s + KROWS, BCOLS:BANDC] = Af[t][:, osl]
        in_maps.append({
            "blob0b": np.ascontiguousarray(blob[0, :, 0:BCOLS]),
            "blob0a": np.ascontiguousarray(blob[0, :, BCOLS:BANDC]),
            "blob12": np.ascontiguousarray(blob[1:]),
        })
    return in_maps


def _gather(results, hidx, best, action_logits, full):
    k = len(hidx)
    lh_ = best[hidx][:, None] * action_logits[hidx]           # [k, A]
    big = np.stack([np.asarray(r["out"]) for r in results])   # [8,3,128,4,188]
    g = big.reshape(NCORES, NBAND, 2, A, TPB, OPC)
    g = g.transpose(1, 4, 2, 0, 5, 3)                         # b,s,pr,c,o,a
    g = g.reshape(KMAX, NO, A)[:k, :N, :].astype(np.float32)
    full[hidx] = g * lh_[:, None, :]


_NC_CACHE = {}


def _build_nc():
    if "nc" in _NC_CACHE:
        return _NC_CACHE["nc"]
    import concourse.bacc as bacc
    import concourse.mybir as mybir
    from concourse.tile import TileContext

    f32 = mybir.dt.float32
    f16 = mybir.dt.float16
    nc = bacc.Bacc()
    blob0b_d = nc.dram_tensor("blob0b", [128, BCOLS], f16, kind="ExternalInput")
    blob0a_d = nc.dram_tensor("blob0a", [128, OPC], f16, kind="ExternalInput")
    blob12_d = nc.dram_tensor(
        "blob12", [NBAND - 1, 128, BANDC], f16, kind="ExternalInput"
    )
    out_d = nc.dram_tensor(
        "out", [NBAND, 128, TPB, OPC], f16, kind="ExternalOutput"
    )

    with TileContext(nc) as tc:
        with (
            tc.tile_pool(name="inp", bufs=3) as ip,
            tc.tile_pool(name="ps", bufs=2, space="PSUM") as pp,
            tc.tile_pool(name="ob", bufs=3) as ob,
        ):
            blobs = []
            for b in range(NBAND):
                blob = ip.tile([128, BANDC], f16, tag="blob")
                if b == 0:
                    nc.scalar.dma_start(blob[:, 0:BCOLS], blob0b_d[:])
                    nc.sync.dma_start(blob[:, BCOLS:BANDC], blob0a_d[:])
                elif b == 1:
                    nc.sync.dma_start(blob[:], blob12_d[b - 1])
                else:
                    nc.gpsimd.dma_start(blob[:], blob12_d[b - 1])
                blobs.append(blob)
            for b in range(NBAND):
                blob = blobs[b]
                ps = pp.tile([128, TPB, 512], f32, tag="ps")
                for s in range(TPB):
                    nc.tensor.matmul(
                        ps[:, s, 0:OPC],
                        blob[32 * s:32 * s + KROWS, 0:BCOLS],
                        blob[32 * s:32 * s + KROWS, BCOLS:BANDC],
                        start=True, stop=True,
                        tile_position=(32 * s, 0),
                    )
                ot = ob.tile([128, TPB, OPC], f16, tag="ot")
                nc.scalar.activation(
                    ot[:], ps[:, :, 0:OPC],
                    mybir.ActivationFunctionType.Exp, scale=float(SCALE),
                )
                if b == NBAND - 1:
                    nc.sync.dma_start(out_d[b][:, 0:2], ot[:, 0:2])
                    nc.scalar.dma_start(out_d[b][:, 2:4], ot[:, 2:4])
                else:
                    nc.sync.dma_start(out_d[b], ot[:])
    nc.finalize()
    _NC_CACHE["nc"] = nc
    return nc


def _run_sim(in_maps):
    results = []
    for m in in_maps:
        b0 = np.concatenate([m["blob0b"], m["blob0a"]], axis=1)
        blob = np.concatenate([b0[None], m["blob12"]], axis=0)
        out = np.zeros((NBAND, 128, TPB, OPC), np.float16)
        for b in range(NBAND):
            for s in range(TPB):
                Bm = blob[b, 32 * s:32 * s + KROWS, 0:BCOLS].astype(np.float32)
                Am = blob[b, 32 * s:32 * s + KROWS, BCOLS:BANDC].astype(
                    np.float32
                )
                S = Bm.T @ Am
                out[b, :, s, :] = np.exp(
                    np.minimum(SCALE * S, 80.0)
                ).astype(np.float16)
        results.append({"out": out})
    return results


def kernel(action_logits, target_mean, bbox, scores):
    action_logits = np.asarray(action_logits, np.float32)
    target_mean = np.asarray(target_mean, np.float32)
    bbox = np.asarray(bbox, np.float32)
    scores = np.asarray(scores, np.float32)

    person, best, w, h, cx, cy, cx_p, cy_p, lw_p, lh_p, lno_p = _obj_arrays(
        bbox, scores
    )
    obj_arr = (cx_p, cy_p, lw_p, lh_p, lno_p)
    hidx_all = np.where(person)[0]

    full = np.zeros((N, N, A), np.float32)
    kernel.last_run = None
    for b0 in range(0, len(hidx_all), KMAX):
        hidx = hidx_all[b0:b0 + KMAX]
        in_maps = _host_prep(hidx, best, w, h, cx, cy, obj_arr, target_mean)
        if os.environ.get("KERNEL_SIM") == "1":
            results = _run_sim(in_maps)
        else:
            from concourse.bass_utils import run_bass_kernel_spmd
            nc = _build_nc()
            kw = {}
            if os.environ.get("KERNEL_TRACE") == "1":
                kw = dict(trace=True, trace_cores=list(range(NCORES)))
            r = run_bass_kernel_spmd(
                nc, in_maps, core_ids=list(range(NCORES)), **kw
            )
            results = r.results
            kernel.last_run = r
        _gather(results, hidx, best, action_logits, full)
    return full
